# revision 20
# baseline (speedup 1.0000x reference)
"""Trainium2 Bass kernel for BlankEmbedding (embedding lookup + blank shift-accumulate).

Reference semantics:
    out = emb[x]                               # [B, S, D]
    preblank[s] = (x[s+1]==BLANK) & (x[s]!=BLANK)   (per row; preblank[S-1]=0)
    out[s] += sum_{k=1..3} preblank[s-k] * emb[x[s-k]]   (zero-pad at row start)

Strategy: data-parallel over the 16384 flattened tokens, 2048 per core.
Each core holds the full table in DRAM (converted to bf16 on host; the
2e-2 relative-error budget dwarfs bf16's 2^-9 rounding) and gathers its
2048 rows with per-partition-index indirect DMAs (16 instructions of
[128, DIM] with token t = 128*i + p; SWDGE indirect DMA is limited to
128 indices per instruction on HW — verified: extra offset columns are
ignored and the per-partition transfer just extends contiguously — and
costs ~1.1us of GPSIMD descriptor-generation each, so the gen chain
~= the 8.4 MB/core DMA roofline and the count is kept minimal).

Gathered tiles are stored straight back to DRAM in bf16. The blank
shift-accumulate is applied as sparse corrections: blanks occur with
probability 1/50257 per token, so the host scans x, finds the affected
positions (<= 3 per blank run), and the kernel build bakes one full
[128, DIM] vector-add per corrected tile, sourced from an extra
per-partition indirect-gather column dedicated to that tile. Engines
can't address single partitions at arbitrary bases, so non-corrected
partitions (and cores without corrections in that tile — all cores run
the same SPMD program) point their gather offset at an all-zero row
appended to the table (index VOCAB), making the add a no-op there. The
compiled graph is cached keyed on the correction structure; a
different blank pattern just triggers a rebuild. Tile dependency
tracking orders gather -> add -> store automatically. The host upcasts
the returned bf16 output to f32.
"""

import numpy as np
import ml_dtypes

VOCAB = 50257
DIM = 1024
BLANK = 100
N_BLANKS = 3
B, S = 4, 4096
N_CORES = 8
TOK = B * S                  # 16384 flattened tokens
TPC = TOK // N_CORES         # 2048 tokens per core
P = 128                      # SBUF partitions
NT = TPC // P                # 16 tiles per core

_CACHE = {}


def _corrections(x):
    """Global correction columns from the token stream.

    Returns (groups, srcs): groups[t] = tile index c_t — correction
    column t is applied as one full [128, DIM] add onto tile c_t (the
    engines can't address single partitions at arbitrary bases, so
    non-corrected partitions point at the zero row instead). srcs[core]
    = {(column t, partition p): src token id} for the slots that core
    actually uses; a repeated (t, p) collision within a core opens
    another column for the same tile.
    """
    xb = np.asarray(x).reshape(B, S)
    is_blank = xb == BLANK
    prev = np.zeros_like(is_blank)
    prev[:, 1:] = is_blank[:, :-1]
    is_first = is_blank & ~prev
    is_pre = np.zeros_like(is_blank)
    is_pre[:, :-1] = is_first[:, 1:]  # position right before a blank run
    groups = []
    srcs = [dict() for _ in range(N_CORES)]
    for b, q in zip(*np.nonzero(is_pre)):
        src = int(xb[b, q])
        for k in range(1, N_BLANKS + 1):
            if q + k >= S:
                break
            d = b * S + q + k
            core, r = divmod(d, TPC)
            c, p = divmod(r, P)
            for t, ct in enumerate(groups):
                if ct == c and (t, p) not in srcs[core]:
                    srcs[core][(t, p)] = src
                    break
            else:
                groups.append(c)
                srcs[core][(len(groups) - 1, p)] = src
    return groups, srcs


def _build_nc(groups):
    from concourse import bacc, mybir, tile
    import concourse.bass as bass

    ncg = len(groups)
    nc = bacc.Bacc(
        "TRN2", target_bir_lowering=False, debug=False, num_devices=N_CORES
    )
    i32 = mybir.dt.int32
    bf16 = mybir.dt.bfloat16
    NCOL = NT + ncg

    ix_dram = nc.dram_tensor("ix_cols", [P, NCOL], i32, kind="ExternalInput")
    emb = nc.dram_tensor("emb", [VOCAB + 1, DIM], bf16, kind="ExternalInput")
    out = nc.dram_tensor("out", [TPC, DIM], bf16, kind="ExternalOutput")

    with tile.TileContext(nc) as tc:
        with tc.tile_pool(name="sbuf", bufs=1) as pool:
            # scalar (Activation) exits the framework prologue ~1us before
            # sync, so its HWDGE load makes the offsets available soonest;
            # the gather chain start is gated on this load's completion
            ix_all = pool.tile([P, NCOL], i32)
            nc.scalar.dma_start(out=ix_all[:], in_=ix_dram[:])

            # correction-source rows first (tiny, gates the adds below)
            corr_g = []
            for t in range(ncg):
                ct = pool.tile([P, DIM], bf16, name=f"corr{t}", tag="corr",
                               bufs=max(ncg, 1))
                nc.gpsimd.indirect_dma_start(
                    out=ct[:], out_offset=None, in_=emb[:],
                    in_offset=bass.IndirectOffsetOnAxis(
                        ap=ix_all[:, NT + t : NT + t + 1], axis=0
                    ),
                )
                corr_g.append(ct)

            # main gathers; correction adds folded in; direct bf16 stores
            adds = {}  # tile c -> [column t]
            for t, c in enumerate(groups):
                adds.setdefault(c, []).append(t)
            for j in range(NT):
                gt = pool.tile([P, DIM], bf16, name=f"g{j}", tag="g", bufs=NT)
                nc.gpsimd.indirect_dma_start(
                    out=gt[:], out_offset=None, in_=emb[:],
                    in_offset=bass.IndirectOffsetOnAxis(
                        ap=ix_all[:, j : j + 1], axis=0
                    ),
                )
                for t in adds.get(j, ()):
                    nc.vector.tensor_tensor(
                        out=gt[:], in0=gt[:], in1=corr_g[t][:],
                        op=mybir.AluOpType.add,
                    )
                nc.sync.dma_start(out=out[P * j : P * (j + 1), :], in_=gt[:])

    nc.compile()
    return nc


def _corr_key(groups):
    return tuple(groups)


def get_nc(groups):
    key = _corr_key(groups)
    if _CACHE.get("key") != key:
        _CACHE["nc"] = _build_nc(groups)
        _CACHE["key"] = key
    return _CACHE["nc"]


def _emb_ext(emb_table):
    """bf16 table with an appended all-zero row (index VOCAB)."""
    e = np.zeros((VOCAB + 1, DIM), dtype=ml_dtypes.bfloat16)
    e[:VOCAB] = np.asarray(emb_table, dtype=np.float32).astype(
        ml_dtypes.bfloat16
    )
    return e


def shard_inputs(x, emb_table, groups, srcs):
    """Build per-core in_maps from full inputs."""
    ncg = len(groups)
    flat = np.ascontiguousarray(np.asarray(x).astype(np.int32).reshape(-1))
    emb_bf16 = _emb_ext(emb_table)
    in_maps = []
    for c in range(N_CORES):
        ix_cols = np.zeros((P, NT + ncg), dtype=np.int32)
        # tile layout: token t = 128*i + p -> column i, partition p
        ix_cols[:, :NT] = flat[c * TPC : (c + 1) * TPC].reshape(NT, P).T
        ix_cols[:, NT:] = VOCAB  # default: all-zero row -> add is a no-op
        for (t, p), src in srcs[c].items():
            ix_cols[p, NT + t] = src
        in_maps.append({"ix_cols": ix_cols, "emb": emb_bf16})
    return in_maps


def assemble_output(results):
    parts = [results[c]["out"] for c in range(N_CORES)]
    return np.concatenate(parts, axis=0).astype(np.float32).reshape(B, S, DIM)


def kernel(x, emb_table):
    from concourse.bass_utils import run_bass_kernel_spmd

    groups, srcs = _corrections(x)
    nc = get_nc(groups)
    in_maps = shard_inputs(x, emb_table, groups, srcs)
    res = run_bass_kernel_spmd(nc, in_maps, core_ids=list(range(N_CORES)))
    return assemble_output(res.results)
